# revision 28
# baseline (speedup 1.0000x reference)
"""Bass/Trainium2 kernel for DepthAttentionResidual.

Math (per (b, t) position, S=16 sources, D=2048):
    ss[s]  = sum_d x[s]^2
    qx[s]  = sum_d q[d] * x[s, d]
    score  = qx * rsqrt(ss/D + eps) / sqrt(D)          # keys never materialized
    w      = softmax_s(score)                          # no max-subtract: |score| ~ N(0,1)
    out[d] = sum_s w[s] * x[s, d]

Sharding: data-parallel over (B x T/2) -> 8 cores; each core gets
x_c = source_bank[:, b, half] of shape [16, 512, 2048] (64 MB) and produces
[512, 2048].

v3 tuning (vs the earlier 184 us/core version):
  - no SWDGE DMAs in the steady state: HWDGE (sync/scalar) and SWDGE
    (gpsimd) transfers interleave catastrophically when mixed (2x slowdown
    measured); the f32r weight masks are now written directly by DVE
    (engines can emit f32r, rounding happens in the output path), replacing
    the per-group gpsimd cast-DMAs
  - groups_per_batch=2 (64-row batches): with SBUF capping x tiles at 5
    buffers, smaller batches double the relative prefetch depth, closing
    the 4-16 us DMA stalls at each softmax batch barrier

v4 tuning (for all-8-cores-concurrent execution, where sustained HBM
bandwidth is ~300-310 GB/s/core instead of ~400 GB/s solo):
  - contig=True: the host pre-permutes each core's x slice (pack_x_core)
    so every [128, QT*D] x-tile is one fully contiguous 4 MB DMA (2-D
    access pattern, 32 KB per partition line) instead of 4 strided 1 MB
    DMAs; x_split=1 (one descriptor per tile) beats 2/4-way splits by
    ~18 us in the full kernel
  - groups_per_batch=1 (32-row batches): maximum relative prefetch depth
    against the slower concurrent DMA stream (-13 us vs gn=2)
  - warm=False: the PE HAM warm-keeping dummy matmuls cost more in queue
    sync than they save in f32r spin-up in this regime (-6 us)
  - stats_bufs=8: deeper stats double-buffering decouples batch bi+1's
    ss/qx accumulation from batch bi's softmax consumers (-6 us)
  - out-DMA stays on the ACT queue: moving it to SP contends with the
    x-load queue (+15 us); PE cannot issue DMAs (SP/ACT/gpsimd only)

v5 tuning (ablation-guided: DMA-only floor ~223 us, +stats ~227,
+softmax ~228, full kernel 264 us -> the whole 35 us gap was in the
weighted-sum path):
  - we_merge=True: build the whole [P, QT, TL] weight mask with ONE DVE
    tensor_tensor mult using stride-0 broadcast APs (i32 broadcast over
    qt, wn broadcast over tl) instead of 4 small tensor_scalar ops per
    group.  The 64 tiny ops/rep sat on the critical path softmax ->
    masks -> PE wsum -> x-slot release; merging them puts the full
    kernel AT the DMA floor (-43 us, biggest single win this session)
  - po_merge=2048: one [32, 2048] PSUM tile per group (matmul dsts still
    512-wide/bank-aligned), so one PSUM->SBUF copy per group instead of
    4 (-23 us vs po_merge=512 pre-we_merge)
  - pspool_bufs=2 (softmax-chain PSUM double-buffering) measured WORSE;
    keep 1
  - x_half=True: each t-group loads as two [128, 2, D] half-tiles (2 MB
    contiguous DMAs, 10-buffer ring instead of 5x4MB) with qt-outer
    matmul order so half A releases mid-group (-10 us: deeper DMA queue
    wins over descriptor economy at 2 MB granularity)

v6 notes (the x_half ring moved the pure-DMA ablation floor to ~200 us
= 336 GB/s/core; stats+softmax re-exposed ~+24 us on top, wsum ~+18 us):
  - groups_per_batch=2: with the lean v5 wsum path, halving the number
    of softmax chains (8/rep instead of 16) beats gn=1's deeper relative
    prefetch (2-of-3 windows, ~8 us at 45-iter decider)
  - sw_pipe (emit batch bi+1's loads+stats before batch bi's
    softmax+wsum to decouple the in-order DVE/ACT streams from the
    chain's PE round-trips): no reliable gain; kept off
  - x2q (alternating x half-tile loads between the sync and scalar
    HWDGE queues): +90 us, catastrophic - never split one tensor's load
    stream across queues
  - fp8 (float8e4) as the discarded elementwise-out dtype of the
    custom-DVE TTR op: wedges the exec unit
    (NRT_EXEC_UNIT_UNRECOVERABLE) - engines cannot emit fp8 from this
    ucode path; bf16 discard buffers stay

v7 tuning:
  - chain_v2=True: shorten the per-batch softmax chain by pre-summing u
    over quarters on DVE ([P, gn] tensor_reduce), so ONE small [TL, gn]
    PE matmul yields the denominator directly (replaces the [TL, batch]
    pd matmul + PSUM-read tensor_reduce pair); reciprocal reads PSUM.
    Fewer DVE<->PE round-trip stalls in the in-order DVE stream
    (-26 us on the min statistic, never worse on p10)
  - copy_split (ACT+DVE each copy half of po in parallel): worse, off
  - po_merge=1024 with ppool_bufs=3 (PSUM double-buffering): worse than
    po2048 bufs=1 even post-we_merge, off

On-chip layout: each SBUF x-tile is [128, 2048] with partition p = tl*4 + sq,
covering 32 t-rows (tl) x 4 sources (sq).  A "t-group" of 32 rows therefore
spans QT=4 such tiles (source quarters).  Per tile:
  - ScalarE: Square activation with accum_out  -> ss column
  - VectorE: tensor_tensor_reduce(x, q_bcast)  -> qx column
  - softmax over s: partition-group sums via a tiny TensorE matmul
    (indicator I32), a free-dim reduce over quarters, reciprocal, and a
    broadcast-back matmul (indicator J4)
  - weighted sum over s: TensorE matmul with lhsT = w * I32 (sparse weights),
    accumulating over the 4 quarters in PSUM.  A batch of 16 tiles =
    128 t-rows fills one [128, 2048] PSUM tile at 32-aligned offsets ->
    one PSUM->SBUF copy -> one 1 MB DMA out.
"""

import math

import numpy as np

S, B, T, D = 16, 4, 1024, 2048
N_CORES = 8
SQ = 4                    # sources per partition-group
QT = S // SQ              # 4 source-quarters
TL = 32                   # t-rows per x-tile
G = 4                     # t-groups per batch
BATCH = G * QT            # 16 x-tiles per batch -> 128 t-rows
P = 128
T_CORE = (B * T) // N_CORES          # 512 t-rows per core
EPS = float(np.finfo(np.float32).eps)
SCALE = 1.0 / math.sqrt(D)           # TEMPERATURE = 1.0
MM_F32R = True            # f32r matmuls: 1 cyc/row vs 4 for fp32; dst must be partition 0

_module_cache = {}


def pack_x_core(xc):
    """Permute one core's [S, t_core, D] slice to the contiguous-DMA layout
    [t_core//TL, P, QT*D]: group g, partition p = tl*SQ + sq, free = (qt, d)
    with source s = qt*SQ + sq.  Makes each x-tile a single fully-contiguous
    4 MB DMA (32 KB per partition line)."""
    t_core = xc.shape[1]
    xp = xc.reshape(QT, SQ, t_core // TL, TL, D)   # (qt, sq, g, tl, d)
    xp = xp.transpose(2, 3, 1, 0, 4)               # (g, tl, sq, qt, d)
    return np.ascontiguousarray(xp.reshape(t_core // TL, P, QT * D))


def build_module(t_core=T_CORE, x_bufs=None, mm_f32r=MM_F32R, groups_per_batch=2, reps=1, we_engine="dve", copy_dve_mod=4, o_bufs=1, stats_bufs=8, contig=True, x_dma="sync", o_dma="scalar", warm=False, x_split=1, po_merge=2048, ppool_bufs=None, pspool_bufs=1, we_merge=True, ablate=3, x_half=True, dummy_dt=None, x2q=False, sw_pipe=False, copy_split=False, chain_v2=True, chain_v3=False, odma_batch=False, wsum_wide=False):
    import concourse.bass as bass
    import concourse.bacc as bacc
    import concourse.mybir as mybir
    import concourse.tile as tile
    from concourse.dve_ops import TENSOR_TENSOR_REDUCE as TTR_OP

    if x_bufs is None:
        x_bufs = 10 if x_half else 5   # same SBUF bytes: 10x16KB vs 5x32KB
    if sw_pipe and o_bufs < 2:
        o_bufs = 2                     # batch bi-1's ostage alive during bi

    fp32 = mybir.dt.float32
    f32r = mybir.dt.float32r
    gn = groups_per_batch
    batch = gn * QT
    rows_per_batch = TL * gn
    n_batches = t_core // rows_per_batch
    assert n_batches * rows_per_batch == t_core

    x_dt = f32r if mm_f32r else fp32
    nc = bacc.Bacc(None)
    if contig:
        x_h = nc.declare_dram_parameter(
            "x", [t_core // TL, P, QT * D], x_dt, isOutput=False)
    else:
        x_h = nc.declare_dram_parameter("x", [S, t_core, D], x_dt, isOutput=False)
    q_h = nc.declare_dram_parameter("q", [D], fp32, isOutput=False)
    o_h = nc.declare_dram_parameter("out", [t_core, D], fp32, isOutput=True)

    # Indicator matrices for partition-group ops (partition p = tl*4 + sq).
    i32 = np.zeros((P, TL), np.float32)
    i32[np.arange(P), np.arange(P) // SQ] = 1.0        # group-sum over sq
    j4 = np.zeros((TL, P), np.float32)
    j4[np.arange(P) // SQ, np.arange(P)] = 1.0         # broadcast back per group
    i32_h = nc.inline_tensor(i32, name="i32const")
    j4_h = nc.inline_tensor(j4, name="j4const")
    # same-tl-group indicator: K[p', p] = 1 iff p'//SQ == p//SQ.  One
    # matmul K^T . us broadcasts the group denominator to all partitions
    # (fuses den matmul + pbc matmul of the softmax chain).
    k128 = (np.arange(P)[:, None] // SQ == np.arange(P)[None, :] // SQ
            ).astype(np.float32)
    k128_h = nc.inline_tensor(k128, name="k128const")

    x_ap = x_h[:]
    q_ap = q_h[:]
    q_bcast = bass.AP(tensor=q_ap.tensor, offset=q_ap.offset, ap=[[0, P], *q_ap.ap])

    AF = mybir.ActivationFunctionType
    OP = mybir.AluOpType

    with tile.TileContext(nc) as tc:
        WE_ENG = {"dve": nc.vector, "pool": nc.gpsimd}[we_engine]
        with (
            tc.tile_pool(name="xpool", bufs=x_bufs) as xpool,
            tc.tile_pool(name="single", bufs=1) as single,
            tc.tile_pool(name="stats", bufs=stats_bufs) as stats,
            tc.tile_pool(name="wepool", bufs=4) as wepool,
            tc.tile_pool(name="opool", bufs=o_bufs) as opool,
            tc.tile_pool(
                name="ppool",
                bufs=(ppool_bufs if ppool_bufs is not None
                      else ({512: 4, 1024: 3, 2048: 1}[po_merge] if mm_f32r else 1)),
                space="PSUM",
            ) as ppool,
            tc.tile_pool(name="pspool", bufs=pspool_bufs, space="PSUM") as pspool,
            tc.tile_pool(name="warmp", bufs=1, space="PSUM") as warmp,
        ):
            q_sb = single.tile([P, D], fp32)
            nc.sync.dma_start(out=q_sb, in_=q_bcast)
            i32_sb = single.tile([P, TL], fp32)
            nc.sync.dma_start(out=i32_sb, in_=i32_h[:])
            j4_sb = single.tile([TL, P], fp32)
            nc.sync.dma_start(out=j4_sb, in_=j4_h[:])
            k128_sb = single.tile([P, P], fp32)
            nc.sync.dma_start(out=k128_sb, in_=k128_h[:])
            eps_sb = single.tile([P, 1], fp32)
            nc.vector.memset(eps_sb, EPS)
            # discarded elementwise outputs (only accum_out is consumed);
            # a narrow dtype shrinks their SBUF footprint
            _gdt = mybir.dt.bfloat16 if dummy_dt is None else getattr(mybir.dt, dummy_dt)
            g_act = single.tile([P, D], _gdt)
            g_dve = single.tile([P, D], _gdt)

            import contextlib
            rep_ctx = (
                tc.For_i(0, reps, 1) if reps > 1 else contextlib.nullcontext()
            )
            with rep_ctx:
                def emit_load_stats(bi):
                    ss_col = stats.tile([P, batch], fp32, tag="ss")
                    qx_col = stats.tile([P, batch], fp32, tag="qx")
                    ostage = opool.tile([rows_per_batch, D], fp32, tag="os")
                    xt = []
                    for g in range(gn):
                        t0 = bi * rows_per_batch + g * TL
                        if x_half:
                            assert contig and x_split == 1
                            xh = []
                            for hi in range(2):
                                xsh = xpool.tile([P, 2, D], x_dt, tag="x")
                                _xe = nc.scalar if (x2q and hi == 1) else nc.sync
                                _xe.dma_start(
                                    out=xsh,
                                    in_=x_ap[
                                        t0 // TL, :,
                                        hi * 2 * D : (hi + 1) * 2 * D,
                                    ].rearrange("p (q d) -> p q d", q=2),
                                )
                                xh.append(xsh)
                            xt.append(xh)
                            for qt in range(QT):
                                if ablate < 1:
                                    continue
                                j = g * QT + qt
                                xsl = xh[qt // 2]
                                xsl_f = xsl.bitcast(fp32) if mm_f32r else xsl
                                nc.scalar.activation(
                                    out=g_act, in_=xsl_f[:, qt % 2, :],
                                    func=AF.Square,
                                    accum_out=ss_col[:, j : j + 1],
                                )
                                nc.vector._custom_dve(
                                    TTR_OP, out=g_dve,
                                    in0=xsl_f[:, qt % 2, :], in1=q_sb,
                                    s0=0.0, s1=1.0,
                                    accum_out=qx_col[:, j : j + 1],
                                )
                            continue
                        xs = xpool.tile([P, QT, D], x_dt, tag="x")
                        if contig:
                            # host-pre-permuted layout (pack_x_core): fully
                            # contiguous partition lines.  x_split=1 -> one
                            # 4MB DMA per tile; x_split=4 -> 4x 1MB DMAs
                            # (more SDMA engines in flight)
                            XDMA = {"sync": nc.sync, "tensor": nc.tensor}[x_dma]
                            if x_split == 1:
                                XDMA.dma_start(
                                    out=xs,
                                    in_=x_ap[t0 // TL].rearrange(
                                        "p (q d) -> p q d", q=QT),
                                )
                            else:
                                step = QT // x_split
                                for si in range(x_split):
                                    XDMA.dma_start(
                                        out=xs[:, si * step : (si + 1) * step, :],
                                        in_=x_ap[
                                            t0 // TL, :,
                                            si * step * D : (si + 1) * step * D,
                                        ].rearrange("p (q d) -> p q d", q=step),
                                    )
                        else:
                            # 4 x 1MB DMAs per t-group into one [P, QT, D]
                            # tile (a single 4MB DMA needs a 4-D access
                            # pattern, which the DMA AP balancer rejects)
                            for qt in range(QT):
                                src = x_ap[
                                    qt * SQ : (qt + 1) * SQ, t0 : t0 + TL, :
                                ].rearrange("s tl d -> tl s d")
                                nc.sync.dma_start(out=xs[:, qt, :], in_=src)
                        xt.append(xs)
                        xs_f = xs.bitcast(fp32) if mm_f32r else xs
                        for qt in range(QT):
                            if ablate < 1:
                                continue
                            j = g * QT + qt
                            nc.scalar.activation(
                                out=g_act, in_=xs_f[:, qt, :], func=AF.Square,
                                accum_out=ss_col[:, j : j + 1],
                            )
                            # ISA TENSOR_TENSOR_REDUCE crashes at runtime here;
                            # the custom-DVE ucode variant works.
                            nc.vector._custom_dve(
                                TTR_OP, out=g_dve, in0=xs_f[:, qt, :], in1=q_sb,
                                s0=0.0, s1=1.0,
                                accum_out=qx_col[:, j : j + 1],
                            )
                            # 1-row dummy matmul tied to this load keeps the
                            # PE clock-gate (HAM) warm between real bursts
                            if warm:
                                wpo = warmp.tile([TL, 1], fp32, tag="wp")
                                nc.tensor.matmul(
                                    wpo, i32_sb, xs_f[:, qt, 0:1],
                                    start=True, stop=True,
                                )

                    return dict(bi=bi, ss_col=ss_col, qx_col=qx_col,
                                ostage=ostage, xt=xt)

                def emit_tail(st):
                    if ablate < 2:
                        return
                    bi = st["bi"]; ss_col = st["ss_col"]
                    qx_col = st["qx_col"]; ostage = st["ostage"]; xt = st["xt"]
                    # --- batch softmax over the 16 sources (per t-row) ---
                    # rsqrt(ms+eps) via Newton on DVE: avoids the Sqrt ACT
                    # table, so the only table set loaded is exp_and_friends
                    v = stats.tile([P, batch], fp32, tag="v")
                    nc.vector.tensor_scalar(
                        out=v, in0=ss_col, scalar1=1.0 / D, scalar2=EPS,
                        op0=OP.mult, op1=OP.add,
                    )
                    y = stats.tile([P, batch], fp32, tag="y")
                    nc.vector.tensor_scalar(
                        out=y, in0=v, scalar1=-0.5, scalar2=1.5,
                        op0=OP.mult, op1=OP.add,
                    )
                    # 1 Newton step suffices: v = ms+eps concentrates in
                    # [0.85, 1.15] (D=2048), linear init err ~1%, one step ~1e-4
                    for _ in range(1):
                        y2 = stats.tile([P, batch], fp32, tag="y2")
                        nc.vector.tensor_mul(y2, y, y)
                        vy2 = stats.tile([P, batch], fp32, tag="vy2")
                        nc.vector.tensor_mul(vy2, v, y2)
                        h = stats.tile([P, batch], fp32, tag="h")
                        nc.vector.tensor_scalar(
                            out=h, in0=vy2, scalar1=-0.5, scalar2=1.5,
                            op0=OP.mult, op1=OP.add,
                        )
                        yn = stats.tile([P, batch], fp32, tag="yn")
                        nc.vector.tensor_mul(yn, y, h)
                        y = yn

                    sc = stats.tile([P, batch], fp32, tag="sc")
                    nc.vector.tensor_mul(sc, qx_col, y)
                    u = stats.tile([P, batch], fp32, tag="u")
                    nc.scalar.activation(out=u, in_=sc, func=AF.Exp, scale=SCALE)

                    if chain_v3:
                        # single PE trip: K^T . us gives the group denom
                        # already broadcast to every partition; recip it
                        # straight out of PSUM into SBUF
                        us = stats.tile([P, gn], fp32, tag="us")
                        nc.vector.tensor_reduce(
                            out=us,
                            in_=u.rearrange("p (g qt) -> p g qt", qt=QT),
                            axis=mybir.AxisListType.X,
                            op=OP.add,
                        )
                        pden = pspool.tile([P, gn], fp32, tag="pd")
                        nc.tensor.matmul(pden, k128_sb, us, start=True, stop=True)
                        bc_sb = stats.tile([P, gn], fp32, tag="bc")
                        nc.vector.reciprocal(out=bc_sb, in_=pden)
                        wn = stats.tile([P, batch], fp32, tag="wn")
                        for g in range(gn):
                            nc.vector.tensor_scalar(
                                out=wn[:, g * QT : (g + 1) * QT],
                                in0=u[:, g * QT : (g + 1) * QT],
                                scalar1=bc_sb[:, g : g + 1],
                                scalar2=None,
                                op0=OP.mult,
                            )
                    elif chain_v2:
                        # pre-sum u over quarters on DVE, then ONE small
                        # matmul gives the denominator; recip reads PSUM
                        us = stats.tile([P, gn], fp32, tag="us")
                        nc.vector.tensor_reduce(
                            out=us,
                            in_=u.rearrange("p (g qt) -> p g qt", qt=QT),
                            axis=mybir.AxisListType.X,
                            op=OP.add,
                        )
                        pd = pspool.tile([TL, gn], fp32, tag="pd")
                        nc.tensor.matmul(pd, i32_sb, us, start=True, stop=True)
                        rd = stats.tile([TL, gn], fp32, tag="rd")
                        nc.vector.reciprocal(out=rd, in_=pd)
                    else:
                        pd = pspool.tile([TL, batch], fp32, tag="pd")
                        nc.tensor.matmul(pd, i32_sb, u, start=True, stop=True)
                        dsum = stats.tile([TL, gn], fp32, tag="dsum")
                        nc.vector.tensor_reduce(
                            out=dsum,
                            in_=pd.rearrange("p (g qt) -> p g qt", qt=QT),
                            axis=mybir.AxisListType.X,
                            op=OP.add,
                        )
                        rd = stats.tile([TL, gn], fp32, tag="rd")
                        nc.vector.reciprocal(out=rd, in_=dsum)
                    if not chain_v3:
                        pbc = pspool.tile([P, gn], fp32, tag="pbc")
                        nc.tensor.matmul(pbc, j4_sb, rd, start=True, stop=True)
                        bc_sb = stats.tile([P, gn], fp32, tag="bc")
                        nc.vector.tensor_copy(bc_sb, pbc)
                        wn = stats.tile([P, batch], fp32, tag="wn")
                        for g in range(gn):
                            nc.vector.tensor_scalar(
                                out=wn[:, g * QT : (g + 1) * QT],
                                in0=u[:, g * QT : (g + 1) * QT],
                                scalar1=bc_sb[:, g : g + 1],
                                scalar2=None,
                                op0=OP.mult,
                            )

                    if ablate < 3:
                        return
                    if wsum_wide:
                        # fp32 wsum matmuls allow partition-offset dsts:
                        # all gn groups accumulate into ONE
                        # [rows_per_batch, D] PSUM tile -> one full-width
                        # copy + one out-DMA per batch
                        pow_ = ppool.tile([rows_per_batch, D], fp32, tag="po")
                        for g in range(gn):
                            we_r = wepool.tile([P, QT, TL], fp32, tag="wea")
                            i32_ap = i32_sb[:, :]
                            i32_b = bass.AP(
                                tensor=i32_ap.tensor, offset=i32_ap.offset,
                                ap=[i32_ap.ap[0], [0, QT], i32_ap.ap[1]])
                            wn_ap = wn[:, g * QT : (g + 1) * QT]
                            wn_b = bass.AP(
                                tensor=wn_ap.tensor, offset=wn_ap.offset,
                                ap=[*wn_ap.ap, [0, TL]])
                            nc.vector.tensor_mul(we_r, i32_b, wn_b)
                            for qt in range(QT):
                                if x_half:
                                    rhs = xt[g][qt // 2][:, qt % 2, :]
                                else:
                                    rhs = xt[g][:, qt, :]
                                rhs = rhs.bitcast(fp32) if mm_f32r else rhs
                                for c1 in range(0, D, 512):
                                    nc.tensor.matmul(
                                        pow_[g * TL : (g + 1) * TL,
                                             c1 : c1 + 512],
                                        we_r[:, qt, :],
                                        rhs[:, c1 : c1 + 512],
                                        start=(qt == 0), stop=(qt == QT - 1),
                                    )
                        nc.scalar.copy(out=ostage, in_=pow_)
                        ODMA = {"scalar": nc.scalar, "tensor": nc.tensor,
                                "sync": nc.sync}[o_dma]
                        ODMA.dma_start(
                            out=o_h[bi * rows_per_batch :
                                    (bi + 1) * rows_per_batch, :],
                            in_=ostage,
                        )
                        return
                    # weighted sum on PE, PSUM-accumulated over quarters;
                    # per-group weights/copies/out-DMAs release x-slots early
                    for g in range(gn):
                        we_dt = f32r if mm_f32r else fp32
                        we_r = wepool.tile([P, QT, TL], we_dt, tag="wea")
                        if we_merge:
                            i32_ap = i32_sb[:, :]
                            i32_b = bass.AP(
                                tensor=i32_ap.tensor, offset=i32_ap.offset,
                                ap=[i32_ap.ap[0], [0, QT], i32_ap.ap[1]])
                            wn_ap = wn[:, g * QT : (g + 1) * QT]
                            wn_b = bass.AP(
                                tensor=wn_ap.tensor, offset=wn_ap.offset,
                                ap=[*wn_ap.ap, [0, TL]])
                            nc.vector.tensor_mul(we_r, i32_b, wn_b)
                        else:
                            for qt in range(QT):
                                j = g * QT + qt
                                WE_ENG.tensor_scalar_mul(
                                    we_r[:, qt, :], i32_sb, wn[:, j : j + 1]
                                )
                        if mm_f32r:
                            # po_merge: width of each PSUM staging tile.
                            # matmul dsts stay 512-wide (one PSUM bank);
                            # wider po tiles mean fewer PSUM->SBUF copies
                            # (less ACT/DVE instruction overhead).
                            for ci, c0 in enumerate(range(0, D, po_merge)):
                                po = ppool.tile([TL, po_merge], fp32, tag="po")
                                if x_half:
                                    # qt-outer: half-tile A (qt 0,1) is fully
                                    # consumed half-way through the group
                                    for qt in range(QT):
                                        rhs = xt[g][qt // 2][:, qt % 2, :]
                                        for c1 in range(0, po_merge, 512):
                                            nc.tensor.matmul(
                                                po[:, c1 : c1 + 512],
                                                we_r[:, qt, :],
                                                rhs[:, c0 + c1 : c0 + c1 + 512],
                                                start=(qt == 0), stop=(qt == QT - 1),
                                            )
                                else:
                                    for c1 in range(0, po_merge, 512):
                                        for qt in range(QT):
                                            nc.tensor.matmul(
                                                po[:, c1 : c1 + 512],
                                                we_r[:, qt, :],
                                                xt[g][:, qt, c0 + c1 : c0 + c1 + 512],
                                                start=(qt == 0), stop=(qt == QT - 1),
                                            )
                                dst = ostage[
                                    g * TL : (g + 1) * TL, c0 : c0 + po_merge]
                                if copy_split:
                                    hw = po_merge // 2
                                    nc.scalar.copy(
                                        out=dst[:, :hw], in_=po[:, :hw])
                                    nc.vector.tensor_copy(
                                        dst[:, hw:], po[:, hw:])
                                # ACT:DVE split - DVE is the busier engine
                                elif (g * 4 + ci) % copy_dve_mod != copy_dve_mod - 1:
                                    nc.scalar.copy(out=dst, in_=po)
                                else:
                                    nc.vector.tensor_copy(dst, po)
                        else:
                            po = ppool.tile([TL, D], fp32, tag="po")
                            for qt in range(QT):
                                for c0 in range(0, D, 512):
                                    nc.tensor.matmul(
                                        po[:, c0 : c0 + 512],
                                        we_r[:, qt, :],
                                        xt[g][:, qt, c0 : c0 + 512],
                                        start=(qt == 0), stop=(qt == QT - 1),
                                    )
                            nc.scalar.copy(
                                out=ostage[g * TL : (g + 1) * TL, :], in_=po
                            )
                        ODMA = {"scalar": nc.scalar, "tensor": nc.tensor, "sync": nc.sync}[o_dma]
                        if not odma_batch:
                            ODMA.dma_start(
                                out=o_h[
                                    bi * rows_per_batch + g * TL :
                                    bi * rows_per_batch + (g + 1) * TL, :
                                ],
                                in_=ostage[g * TL : (g + 1) * TL, :],
                            )
                    if odma_batch:
                        ODMA = {"scalar": nc.scalar, "tensor": nc.tensor, "sync": nc.sync}[o_dma]
                        ODMA.dma_start(
                            out=o_h[bi * rows_per_batch :
                                    (bi + 1) * rows_per_batch, :],
                            in_=ostage,
                        )

                # sw_pipe: emit batch bi+1's loads+stats BEFORE batch bi's
                # softmax+wsum so the in-order DVE/ACT streams have ready
                # work queued ahead of the chain's PE round-trip stalls
                if sw_pipe:
                    pend = None
                    for bi in range(n_batches):
                        cur = emit_load_stats(bi)
                        if pend is not None:
                            emit_tail(pend)
                        pend = cur
                    emit_tail(pend)
                else:
                    for bi in range(n_batches):
                        emit_tail(emit_load_stats(bi))

    nc.compile()
    return nc


CONTIG = True


def _get_module():
    key = (T_CORE, MM_F32R, CONTIG)
    if key not in _module_cache:
        _module_cache[key] = build_module(contig=CONTIG)
    return _module_cache[key]


def core_input(x, c):
    """Per-core "x" array for core c from the full [S, B, T, D] bank."""
    b, h = c // 2, c % 2
    xc = x[:, b, h * T_CORE : (h + 1) * T_CORE, :]
    if CONTIG:
        return pack_x_core(xc)
    return np.ascontiguousarray(xc)


def _run(layer_query, source_bank, **spmd_kwargs):
    from concourse.bass_utils import run_bass_kernel_spmd

    q = np.ascontiguousarray(np.asarray(layer_query, dtype=np.float32))
    x = np.asarray(source_bank, dtype=np.float32)
    assert x.shape == (S, B, T, D)

    nc = _get_module()
    in_maps = []
    for c in range(N_CORES):
        in_maps.append({"x": core_input(x, c), "q": q})

    res = run_bass_kernel_spmd(nc, in_maps, core_ids=list(range(N_CORES)), **spmd_kwargs)
    full = np.empty((B, T, D), dtype=np.float32)
    for c in range(N_CORES):
        b, h = c // 2, c % 2
        full[b, h * T_CORE : (h + 1) * T_CORE, :] = res.results[c]["out"]
    return full, res


def kernel(layer_query, source_bank, num_sources=None):
    full, _ = _run(layer_query, source_bank)
    return full



# revision 29
# speedup vs baseline: 1.1535x; 1.1535x over previous
"""Bass/Trainium2 kernel for DepthAttentionResidual.

Math (per (b, t) position, S=16 sources, D=2048):
    ss[s]  = sum_d x[s]^2
    qx[s]  = sum_d q[d] * x[s, d]
    score  = qx * rsqrt(ss/D + eps) / sqrt(D)          # keys never materialized
    w      = softmax_s(score)                          # no max-subtract: |score| ~ N(0,1)
    out[d] = sum_s w[s] * x[s, d]

Sharding: data-parallel over (B x T/2) -> 8 cores; each core gets
x_c = source_bank[:, b, half] of shape [16, 512, 2048] (64 MB) and produces
[512, 2048].

v3 tuning (vs the earlier 184 us/core version):
  - no SWDGE DMAs in the steady state: HWDGE (sync/scalar) and SWDGE
    (gpsimd) transfers interleave catastrophically when mixed (2x slowdown
    measured); the f32r weight masks are now written directly by DVE
    (engines can emit f32r, rounding happens in the output path), replacing
    the per-group gpsimd cast-DMAs
  - groups_per_batch=2 (64-row batches): with SBUF capping x tiles at 5
    buffers, smaller batches double the relative prefetch depth, closing
    the 4-16 us DMA stalls at each softmax batch barrier

v4 tuning (for all-8-cores-concurrent execution, where sustained HBM
bandwidth is ~300-310 GB/s/core instead of ~400 GB/s solo):
  - contig=True: the host pre-permutes each core's x slice (pack_x_core)
    so every [128, QT*D] x-tile is one fully contiguous 4 MB DMA (2-D
    access pattern, 32 KB per partition line) instead of 4 strided 1 MB
    DMAs; x_split=1 (one descriptor per tile) beats 2/4-way splits by
    ~18 us in the full kernel
  - groups_per_batch=1 (32-row batches): maximum relative prefetch depth
    against the slower concurrent DMA stream (-13 us vs gn=2)
  - warm=False: the PE HAM warm-keeping dummy matmuls cost more in queue
    sync than they save in f32r spin-up in this regime (-6 us)
  - stats_bufs=8: deeper stats double-buffering decouples batch bi+1's
    ss/qx accumulation from batch bi's softmax consumers (-6 us)
  - out-DMA stays on the ACT queue: moving it to SP contends with the
    x-load queue (+15 us); PE cannot issue DMAs (SP/ACT/gpsimd only)

v5 tuning (ablation-guided: DMA-only floor ~223 us, +stats ~227,
+softmax ~228, full kernel 264 us -> the whole 35 us gap was in the
weighted-sum path):
  - we_merge=True: build the whole [P, QT, TL] weight mask with ONE DVE
    tensor_tensor mult using stride-0 broadcast APs (i32 broadcast over
    qt, wn broadcast over tl) instead of 4 small tensor_scalar ops per
    group.  The 64 tiny ops/rep sat on the critical path softmax ->
    masks -> PE wsum -> x-slot release; merging them puts the full
    kernel AT the DMA floor (-43 us, biggest single win this session)
  - po_merge=2048: one [32, 2048] PSUM tile per group (matmul dsts still
    512-wide/bank-aligned), so one PSUM->SBUF copy per group instead of
    4 (-23 us vs po_merge=512 pre-we_merge)
  - pspool_bufs=2 (softmax-chain PSUM double-buffering) measured WORSE;
    keep 1
  - x_half=True: each t-group loads as two [128, 2, D] half-tiles (2 MB
    contiguous DMAs, 10-buffer ring instead of 5x4MB) with qt-outer
    matmul order so half A releases mid-group (-10 us: deeper DMA queue
    wins over descriptor economy at 2 MB granularity)

v6 notes (the x_half ring moved the pure-DMA ablation floor to ~200 us
= 336 GB/s/core; stats+softmax re-exposed ~+24 us on top, wsum ~+18 us):
  - groups_per_batch=2: with the lean v5 wsum path, halving the number
    of softmax chains (8/rep instead of 16) beats gn=1's deeper relative
    prefetch (2-of-3 windows, ~8 us at 45-iter decider)
  - sw_pipe (emit batch bi+1's loads+stats before batch bi's
    softmax+wsum to decouple the in-order DVE/ACT streams from the
    chain's PE round-trips): no reliable gain; kept off
  - x2q (alternating x half-tile loads between the sync and scalar
    HWDGE queues): +90 us, catastrophic - never split one tensor's load
    stream across queues
  - fp8 (float8e4) as the discarded elementwise-out dtype of the
    custom-DVE TTR op: wedges the exec unit
    (NRT_EXEC_UNIT_UNRECOVERABLE) - engines cannot emit fp8 from this
    ucode path; bf16 discard buffers stay

v7 tuning:
  - chain_v2=True: shorten the per-batch softmax chain by pre-summing u
    over quarters on DVE ([P, gn] tensor_reduce), so ONE small [TL, gn]
    PE matmul yields the denominator directly (replaces the [TL, batch]
    pd matmul + PSUM-read tensor_reduce pair); reciprocal reads PSUM.
    Fewer DVE<->PE round-trip stalls in the in-order DVE stream
    (-26 us on the min statistic, never worse on p10)
  - copy_split (ACT+DVE each copy half of po in parallel): worse, off
  - po_merge=1024 with ppool_bufs=3 (PSUM double-buffering): worse than
    po2048 bufs=1 even post-we_merge, off

On-chip layout: each SBUF x-tile is [128, 2048] with partition p = tl*4 + sq,
covering 32 t-rows (tl) x 4 sources (sq).  A "t-group" of 32 rows therefore
spans QT=4 such tiles (source quarters).  Per tile:
  - ScalarE: Square activation with accum_out  -> ss column
  - VectorE: tensor_tensor_reduce(x, q_bcast)  -> qx column
  - softmax over s: partition-group sums via a tiny TensorE matmul
    (indicator I32), a free-dim reduce over quarters, reciprocal, and a
    broadcast-back matmul (indicator J4)
  - weighted sum over s: TensorE matmul with lhsT = w * I32 (sparse weights),
    accumulating over the 4 quarters in PSUM.  A batch of 16 tiles =
    128 t-rows fills one [128, 2048] PSUM tile at 32-aligned offsets ->
    one PSUM->SBUF copy -> one 1 MB DMA out.
"""

import math

import numpy as np

S, B, T, D = 16, 4, 1024, 2048
N_CORES = 8
SQ = 4                    # sources per partition-group
QT = S // SQ              # 4 source-quarters
TL = 32                   # t-rows per x-tile
G = 4                     # t-groups per batch
BATCH = G * QT            # 16 x-tiles per batch -> 128 t-rows
P = 128
T_CORE = (B * T) // N_CORES          # 512 t-rows per core
EPS = float(np.finfo(np.float32).eps)
SCALE = 1.0 / math.sqrt(D)           # TEMPERATURE = 1.0
MM_F32R = True            # f32r matmuls: 1 cyc/row vs 4 for fp32; dst must be partition 0

_module_cache = {}


def pack_x_core(xc):
    """Permute one core's [S, t_core, D] slice to the contiguous-DMA layout
    [t_core//TL, P, QT*D]: group g, partition p = tl*SQ + sq, free = (qt, d)
    with source s = qt*SQ + sq.  Makes each x-tile a single fully-contiguous
    4 MB DMA (32 KB per partition line)."""
    t_core = xc.shape[1]
    xp = xc.reshape(QT, SQ, t_core // TL, TL, D)   # (qt, sq, g, tl, d)
    xp = xp.transpose(2, 3, 1, 0, 4)               # (g, tl, sq, qt, d)
    return np.ascontiguousarray(xp.reshape(t_core // TL, P, QT * D))


def build_module(t_core=T_CORE, x_bufs=None, mm_f32r=MM_F32R, groups_per_batch=2, reps=1, we_engine="dve", copy_dve_mod=4, o_bufs=1, stats_bufs=8, contig=True, x_dma="sync", o_dma="scalar", warm=False, x_split=1, po_merge=2048, ppool_bufs=None, pspool_bufs=1, we_merge=True, ablate=3, x_half=True, dummy_dt=None, x2q=False, sw_pipe=False, copy_split=False, chain_v2=True, chain_v3=False, odma_batch=False, wsum_wide=False):
    import concourse.bass as bass
    import concourse.bacc as bacc
    import concourse.mybir as mybir
    import concourse.tile as tile
    from concourse.dve_ops import TENSOR_TENSOR_REDUCE as TTR_OP

    if x_bufs is None:
        x_bufs = 11 if x_half else 5   # 11x16KB fits in SBUF (~204/208KB)
    if sw_pipe and o_bufs < 2:
        o_bufs = 2                     # batch bi-1's ostage alive during bi

    fp32 = mybir.dt.float32
    f32r = mybir.dt.float32r
    gn = groups_per_batch
    batch = gn * QT
    rows_per_batch = TL * gn
    n_batches = t_core // rows_per_batch
    assert n_batches * rows_per_batch == t_core

    x_dt = f32r if mm_f32r else fp32
    nc = bacc.Bacc(None)
    if contig:
        x_h = nc.declare_dram_parameter(
            "x", [t_core // TL, P, QT * D], x_dt, isOutput=False)
    else:
        x_h = nc.declare_dram_parameter("x", [S, t_core, D], x_dt, isOutput=False)
    q_h = nc.declare_dram_parameter("q", [D], fp32, isOutput=False)
    o_h = nc.declare_dram_parameter("out", [t_core, D], fp32, isOutput=True)

    # Indicator matrices for partition-group ops (partition p = tl*4 + sq).
    i32 = np.zeros((P, TL), np.float32)
    i32[np.arange(P), np.arange(P) // SQ] = 1.0        # group-sum over sq
    j4 = np.zeros((TL, P), np.float32)
    j4[np.arange(P) // SQ, np.arange(P)] = 1.0         # broadcast back per group
    i32_h = nc.inline_tensor(i32, name="i32const")
    j4_h = nc.inline_tensor(j4, name="j4const")
    # same-tl-group indicator: K[p', p] = 1 iff p'//SQ == p//SQ.  One
    # matmul K^T . us broadcasts the group denominator to all partitions
    # (fuses den matmul + pbc matmul of the softmax chain).
    k128 = (np.arange(P)[:, None] // SQ == np.arange(P)[None, :] // SQ
            ).astype(np.float32)
    k128_h = nc.inline_tensor(k128, name="k128const")

    x_ap = x_h[:]
    q_ap = q_h[:]
    q_bcast = bass.AP(tensor=q_ap.tensor, offset=q_ap.offset, ap=[[0, P], *q_ap.ap])

    AF = mybir.ActivationFunctionType
    OP = mybir.AluOpType

    with tile.TileContext(nc) as tc:
        WE_ENG = {"dve": nc.vector, "pool": nc.gpsimd}[we_engine]
        with (
            tc.tile_pool(name="xpool", bufs=x_bufs) as xpool,
            tc.tile_pool(name="single", bufs=1) as single,
            tc.tile_pool(name="stats", bufs=stats_bufs) as stats,
            tc.tile_pool(name="wepool", bufs=4) as wepool,
            tc.tile_pool(name="opool", bufs=o_bufs) as opool,
            tc.tile_pool(
                name="ppool",
                bufs=(ppool_bufs if ppool_bufs is not None
                      else ({512: 4, 1024: 3, 2048: 1}[po_merge] if mm_f32r else 1)),
                space="PSUM",
            ) as ppool,
            tc.tile_pool(name="pspool", bufs=pspool_bufs, space="PSUM") as pspool,
            tc.tile_pool(name="warmp", bufs=1, space="PSUM") as warmp,
        ):
            q_sb = single.tile([P, D], fp32)
            nc.sync.dma_start(out=q_sb, in_=q_bcast)
            i32_sb = single.tile([P, TL], fp32)
            nc.sync.dma_start(out=i32_sb, in_=i32_h[:])
            j4_sb = single.tile([TL, P], fp32)
            nc.sync.dma_start(out=j4_sb, in_=j4_h[:])
            k128_sb = single.tile([P, P], fp32)
            nc.sync.dma_start(out=k128_sb, in_=k128_h[:])
            eps_sb = single.tile([P, 1], fp32)
            nc.vector.memset(eps_sb, EPS)
            # discarded elementwise outputs (only accum_out is consumed);
            # a narrow dtype shrinks their SBUF footprint
            _gdt = mybir.dt.bfloat16 if dummy_dt is None else getattr(mybir.dt, dummy_dt)
            g_act = single.tile([P, D], _gdt)
            g_dve = single.tile([P, D], _gdt)

            import contextlib
            rep_ctx = (
                tc.For_i(0, reps, 1) if reps > 1 else contextlib.nullcontext()
            )
            with rep_ctx:
                def emit_load_stats(bi):
                    ss_col = stats.tile([P, batch], fp32, tag="ss")
                    qx_col = stats.tile([P, batch], fp32, tag="qx")
                    ostage = opool.tile([rows_per_batch, D], fp32, tag="os")
                    xt = []
                    for g in range(gn):
                        t0 = bi * rows_per_batch + g * TL
                        if x_half:
                            assert contig and x_split == 1
                            xh = []
                            for hi in range(2):
                                xsh = xpool.tile([P, 2, D], x_dt, tag="x")
                                _xe = nc.scalar if (x2q and hi == 1) else nc.sync
                                _xe.dma_start(
                                    out=xsh,
                                    in_=x_ap[
                                        t0 // TL, :,
                                        hi * 2 * D : (hi + 1) * 2 * D,
                                    ].rearrange("p (q d) -> p q d", q=2),
                                )
                                xh.append(xsh)
                            xt.append(xh)
                            for qt in range(QT):
                                if ablate < 1:
                                    continue
                                j = g * QT + qt
                                xsl = xh[qt // 2]
                                xsl_f = xsl.bitcast(fp32) if mm_f32r else xsl
                                nc.scalar.activation(
                                    out=g_act, in_=xsl_f[:, qt % 2, :],
                                    func=AF.Square,
                                    accum_out=ss_col[:, j : j + 1],
                                )
                                nc.vector._custom_dve(
                                    TTR_OP, out=g_dve,
                                    in0=xsl_f[:, qt % 2, :], in1=q_sb,
                                    s0=0.0, s1=1.0,
                                    accum_out=qx_col[:, j : j + 1],
                                )
                            continue
                        xs = xpool.tile([P, QT, D], x_dt, tag="x")
                        if contig:
                            # host-pre-permuted layout (pack_x_core): fully
                            # contiguous partition lines.  x_split=1 -> one
                            # 4MB DMA per tile; x_split=4 -> 4x 1MB DMAs
                            # (more SDMA engines in flight)
                            XDMA = {"sync": nc.sync, "tensor": nc.tensor}[x_dma]
                            if x_split == 1:
                                XDMA.dma_start(
                                    out=xs,
                                    in_=x_ap[t0 // TL].rearrange(
                                        "p (q d) -> p q d", q=QT),
                                )
                            else:
                                step = QT // x_split
                                for si in range(x_split):
                                    XDMA.dma_start(
                                        out=xs[:, si * step : (si + 1) * step, :],
                                        in_=x_ap[
                                            t0 // TL, :,
                                            si * step * D : (si + 1) * step * D,
                                        ].rearrange("p (q d) -> p q d", q=step),
                                    )
                        else:
                            # 4 x 1MB DMAs per t-group into one [P, QT, D]
                            # tile (a single 4MB DMA needs a 4-D access
                            # pattern, which the DMA AP balancer rejects)
                            for qt in range(QT):
                                src = x_ap[
                                    qt * SQ : (qt + 1) * SQ, t0 : t0 + TL, :
                                ].rearrange("s tl d -> tl s d")
                                nc.sync.dma_start(out=xs[:, qt, :], in_=src)
                        xt.append(xs)
                        xs_f = xs.bitcast(fp32) if mm_f32r else xs
                        for qt in range(QT):
                            if ablate < 1:
                                continue
                            j = g * QT + qt
                            nc.scalar.activation(
                                out=g_act, in_=xs_f[:, qt, :], func=AF.Square,
                                accum_out=ss_col[:, j : j + 1],
                            )
                            # ISA TENSOR_TENSOR_REDUCE crashes at runtime here;
                            # the custom-DVE ucode variant works.
                            nc.vector._custom_dve(
                                TTR_OP, out=g_dve, in0=xs_f[:, qt, :], in1=q_sb,
                                s0=0.0, s1=1.0,
                                accum_out=qx_col[:, j : j + 1],
                            )
                            # 1-row dummy matmul tied to this load keeps the
                            # PE clock-gate (HAM) warm between real bursts
                            if warm:
                                wpo = warmp.tile([TL, 1], fp32, tag="wp")
                                nc.tensor.matmul(
                                    wpo, i32_sb, xs_f[:, qt, 0:1],
                                    start=True, stop=True,
                                )

                    return dict(bi=bi, ss_col=ss_col, qx_col=qx_col,
                                ostage=ostage, xt=xt)

                def emit_tail(st):
                    if ablate < 2:
                        return
                    bi = st["bi"]; ss_col = st["ss_col"]
                    qx_col = st["qx_col"]; ostage = st["ostage"]; xt = st["xt"]
                    # --- batch softmax over the 16 sources (per t-row) ---
                    # rsqrt(ms+eps) via Newton on DVE: avoids the Sqrt ACT
                    # table, so the only table set loaded is exp_and_friends
                    v = stats.tile([P, batch], fp32, tag="v")
                    nc.vector.tensor_scalar(
                        out=v, in0=ss_col, scalar1=1.0 / D, scalar2=EPS,
                        op0=OP.mult, op1=OP.add,
                    )
                    y = stats.tile([P, batch], fp32, tag="y")
                    nc.vector.tensor_scalar(
                        out=y, in0=v, scalar1=-0.5, scalar2=1.5,
                        op0=OP.mult, op1=OP.add,
                    )
                    # 1 Newton step suffices: v = ms+eps concentrates in
                    # [0.85, 1.15] (D=2048), linear init err ~1%, one step ~1e-4
                    for _ in range(1):
                        y2 = stats.tile([P, batch], fp32, tag="y2")
                        nc.vector.tensor_mul(y2, y, y)
                        vy2 = stats.tile([P, batch], fp32, tag="vy2")
                        nc.vector.tensor_mul(vy2, v, y2)
                        h = stats.tile([P, batch], fp32, tag="h")
                        nc.vector.tensor_scalar(
                            out=h, in0=vy2, scalar1=-0.5, scalar2=1.5,
                            op0=OP.mult, op1=OP.add,
                        )
                        yn = stats.tile([P, batch], fp32, tag="yn")
                        nc.vector.tensor_mul(yn, y, h)
                        y = yn

                    sc = stats.tile([P, batch], fp32, tag="sc")
                    nc.vector.tensor_mul(sc, qx_col, y)
                    u = stats.tile([P, batch], fp32, tag="u")
                    nc.scalar.activation(out=u, in_=sc, func=AF.Exp, scale=SCALE)

                    if chain_v3:
                        # single PE trip: K^T . us gives the group denom
                        # already broadcast to every partition; recip it
                        # straight out of PSUM into SBUF
                        us = stats.tile([P, gn], fp32, tag="us")
                        nc.vector.tensor_reduce(
                            out=us,
                            in_=u.rearrange("p (g qt) -> p g qt", qt=QT),
                            axis=mybir.AxisListType.X,
                            op=OP.add,
                        )
                        pden = pspool.tile([P, gn], fp32, tag="pd")
                        nc.tensor.matmul(pden, k128_sb, us, start=True, stop=True)
                        bc_sb = stats.tile([P, gn], fp32, tag="bc")
                        nc.vector.reciprocal(out=bc_sb, in_=pden)
                        wn = stats.tile([P, batch], fp32, tag="wn")
                        for g in range(gn):
                            nc.vector.tensor_scalar(
                                out=wn[:, g * QT : (g + 1) * QT],
                                in0=u[:, g * QT : (g + 1) * QT],
                                scalar1=bc_sb[:, g : g + 1],
                                scalar2=None,
                                op0=OP.mult,
                            )
                    elif chain_v2:
                        # pre-sum u over quarters on DVE, then ONE small
                        # matmul gives the denominator; recip reads PSUM
                        us = stats.tile([P, gn], fp32, tag="us")
                        nc.vector.tensor_reduce(
                            out=us,
                            in_=u.rearrange("p (g qt) -> p g qt", qt=QT),
                            axis=mybir.AxisListType.X,
                            op=OP.add,
                        )
                        pd = pspool.tile([TL, gn], fp32, tag="pd")
                        nc.tensor.matmul(pd, i32_sb, us, start=True, stop=True)
                        rd = stats.tile([TL, gn], fp32, tag="rd")
                        nc.vector.reciprocal(out=rd, in_=pd)
                    else:
                        pd = pspool.tile([TL, batch], fp32, tag="pd")
                        nc.tensor.matmul(pd, i32_sb, u, start=True, stop=True)
                        dsum = stats.tile([TL, gn], fp32, tag="dsum")
                        nc.vector.tensor_reduce(
                            out=dsum,
                            in_=pd.rearrange("p (g qt) -> p g qt", qt=QT),
                            axis=mybir.AxisListType.X,
                            op=OP.add,
                        )
                        rd = stats.tile([TL, gn], fp32, tag="rd")
                        nc.vector.reciprocal(out=rd, in_=dsum)
                    if not chain_v3:
                        pbc = pspool.tile([P, gn], fp32, tag="pbc")
                        nc.tensor.matmul(pbc, j4_sb, rd, start=True, stop=True)
                        bc_sb = stats.tile([P, gn], fp32, tag="bc")
                        nc.vector.tensor_copy(bc_sb, pbc)
                        wn = stats.tile([P, batch], fp32, tag="wn")
                        for g in range(gn):
                            nc.vector.tensor_scalar(
                                out=wn[:, g * QT : (g + 1) * QT],
                                in0=u[:, g * QT : (g + 1) * QT],
                                scalar1=bc_sb[:, g : g + 1],
                                scalar2=None,
                                op0=OP.mult,
                            )

                    if ablate < 3:
                        return
                    if wsum_wide:
                        # fp32 wsum matmuls allow partition-offset dsts:
                        # all gn groups accumulate into ONE
                        # [rows_per_batch, D] PSUM tile -> one full-width
                        # copy + one out-DMA per batch
                        pow_ = ppool.tile([rows_per_batch, D], fp32, tag="po")
                        for g in range(gn):
                            we_r = wepool.tile([P, QT, TL], fp32, tag="wea")
                            i32_ap = i32_sb[:, :]
                            i32_b = bass.AP(
                                tensor=i32_ap.tensor, offset=i32_ap.offset,
                                ap=[i32_ap.ap[0], [0, QT], i32_ap.ap[1]])
                            wn_ap = wn[:, g * QT : (g + 1) * QT]
                            wn_b = bass.AP(
                                tensor=wn_ap.tensor, offset=wn_ap.offset,
                                ap=[*wn_ap.ap, [0, TL]])
                            nc.vector.tensor_mul(we_r, i32_b, wn_b)
                            for qt in range(QT):
                                if x_half:
                                    rhs = xt[g][qt // 2][:, qt % 2, :]
                                else:
                                    rhs = xt[g][:, qt, :]
                                rhs = rhs.bitcast(fp32) if mm_f32r else rhs
                                for c1 in range(0, D, 512):
                                    nc.tensor.matmul(
                                        pow_[g * TL : (g + 1) * TL,
                                             c1 : c1 + 512],
                                        we_r[:, qt, :],
                                        rhs[:, c1 : c1 + 512],
                                        start=(qt == 0), stop=(qt == QT - 1),
                                    )
                        nc.scalar.copy(out=ostage, in_=pow_)
                        ODMA = {"scalar": nc.scalar, "tensor": nc.tensor,
                                "sync": nc.sync}[o_dma]
                        ODMA.dma_start(
                            out=o_h[bi * rows_per_batch :
                                    (bi + 1) * rows_per_batch, :],
                            in_=ostage,
                        )
                        return
                    # weighted sum on PE, PSUM-accumulated over quarters;
                    # per-group weights/copies/out-DMAs release x-slots early
                    for g in range(gn):
                        we_dt = f32r if mm_f32r else fp32
                        we_r = wepool.tile([P, QT, TL], we_dt, tag="wea")
                        if we_merge:
                            i32_ap = i32_sb[:, :]
                            i32_b = bass.AP(
                                tensor=i32_ap.tensor, offset=i32_ap.offset,
                                ap=[i32_ap.ap[0], [0, QT], i32_ap.ap[1]])
                            wn_ap = wn[:, g * QT : (g + 1) * QT]
                            wn_b = bass.AP(
                                tensor=wn_ap.tensor, offset=wn_ap.offset,
                                ap=[*wn_ap.ap, [0, TL]])
                            nc.vector.tensor_mul(we_r, i32_b, wn_b)
                        else:
                            for qt in range(QT):
                                j = g * QT + qt
                                WE_ENG.tensor_scalar_mul(
                                    we_r[:, qt, :], i32_sb, wn[:, j : j + 1]
                                )
                        if mm_f32r:
                            # po_merge: width of each PSUM staging tile.
                            # matmul dsts stay 512-wide (one PSUM bank);
                            # wider po tiles mean fewer PSUM->SBUF copies
                            # (less ACT/DVE instruction overhead).
                            for ci, c0 in enumerate(range(0, D, po_merge)):
                                po = ppool.tile([TL, po_merge], fp32, tag="po")
                                if x_half:
                                    # qt-outer: half-tile A (qt 0,1) is fully
                                    # consumed half-way through the group
                                    for qt in range(QT):
                                        rhs = xt[g][qt // 2][:, qt % 2, :]
                                        for c1 in range(0, po_merge, 512):
                                            nc.tensor.matmul(
                                                po[:, c1 : c1 + 512],
                                                we_r[:, qt, :],
                                                rhs[:, c0 + c1 : c0 + c1 + 512],
                                                start=(qt == 0), stop=(qt == QT - 1),
                                            )
                                else:
                                    for c1 in range(0, po_merge, 512):
                                        for qt in range(QT):
                                            nc.tensor.matmul(
                                                po[:, c1 : c1 + 512],
                                                we_r[:, qt, :],
                                                xt[g][:, qt, c0 + c1 : c0 + c1 + 512],
                                                start=(qt == 0), stop=(qt == QT - 1),
                                            )
                                dst = ostage[
                                    g * TL : (g + 1) * TL, c0 : c0 + po_merge]
                                if copy_split:
                                    hw = po_merge // 2
                                    nc.scalar.copy(
                                        out=dst[:, :hw], in_=po[:, :hw])
                                    nc.vector.tensor_copy(
                                        dst[:, hw:], po[:, hw:])
                                # ACT:DVE split - DVE is the busier engine
                                elif (g * 4 + ci) % copy_dve_mod != copy_dve_mod - 1:
                                    nc.scalar.copy(out=dst, in_=po)
                                else:
                                    nc.vector.tensor_copy(dst, po)
                        else:
                            po = ppool.tile([TL, D], fp32, tag="po")
                            for qt in range(QT):
                                for c0 in range(0, D, 512):
                                    nc.tensor.matmul(
                                        po[:, c0 : c0 + 512],
                                        we_r[:, qt, :],
                                        xt[g][:, qt, c0 : c0 + 512],
                                        start=(qt == 0), stop=(qt == QT - 1),
                                    )
                            nc.scalar.copy(
                                out=ostage[g * TL : (g + 1) * TL, :], in_=po
                            )
                        ODMA = {"scalar": nc.scalar, "tensor": nc.tensor, "sync": nc.sync}[o_dma]
                        if not odma_batch:
                            ODMA.dma_start(
                                out=o_h[
                                    bi * rows_per_batch + g * TL :
                                    bi * rows_per_batch + (g + 1) * TL, :
                                ],
                                in_=ostage[g * TL : (g + 1) * TL, :],
                            )
                    if odma_batch:
                        ODMA = {"scalar": nc.scalar, "tensor": nc.tensor, "sync": nc.sync}[o_dma]
                        ODMA.dma_start(
                            out=o_h[bi * rows_per_batch :
                                    (bi + 1) * rows_per_batch, :],
                            in_=ostage,
                        )

                # sw_pipe: emit batch bi+1's loads+stats BEFORE batch bi's
                # softmax+wsum so the in-order DVE/ACT streams have ready
                # work queued ahead of the chain's PE round-trip stalls
                if sw_pipe:
                    pend = None
                    for bi in range(n_batches):
                        cur = emit_load_stats(bi)
                        if pend is not None:
                            emit_tail(pend)
                        pend = cur
                    emit_tail(pend)
                else:
                    for bi in range(n_batches):
                        emit_tail(emit_load_stats(bi))

    nc.compile()
    return nc


CONTIG = True


def _get_module():
    key = (T_CORE, MM_F32R, CONTIG)
    if key not in _module_cache:
        _module_cache[key] = build_module(contig=CONTIG)
    return _module_cache[key]


def core_input(x, c):
    """Per-core "x" array for core c from the full [S, B, T, D] bank."""
    b, h = c // 2, c % 2
    xc = x[:, b, h * T_CORE : (h + 1) * T_CORE, :]
    if CONTIG:
        return pack_x_core(xc)
    return np.ascontiguousarray(xc)


def _run(layer_query, source_bank, **spmd_kwargs):
    from concourse.bass_utils import run_bass_kernel_spmd

    q = np.ascontiguousarray(np.asarray(layer_query, dtype=np.float32))
    x = np.asarray(source_bank, dtype=np.float32)
    assert x.shape == (S, B, T, D)

    nc = _get_module()
    in_maps = []
    for c in range(N_CORES):
        in_maps.append({"x": core_input(x, c), "q": q})

    res = run_bass_kernel_spmd(nc, in_maps, core_ids=list(range(N_CORES)), **spmd_kwargs)
    full = np.empty((B, T, D), dtype=np.float32)
    for c in range(N_CORES):
        b, h = c // 2, c % 2
        full[b, h * T_CORE : (h + 1) * T_CORE, :] = res.results[c]["out"]
    return full, res


def kernel(layer_query, source_bank, num_sources=None):
    full, _ = _run(layer_query, source_bank)
    return full

